# revision 1
# baseline (speedup 1.0000x reference)
"""ADGN (antisymmetric DGN) message-passing GNN on 8 TRN2 NeuronCores.

Strategy (self-contained, hardcoded for the nn_ADGN problem):
  - Nodes are sharded at GRAPH boundaries: core c owns graphs [16c, 16c+16)
    and their nodes (batch is sorted, so node ranges are contiguous). Per-core
    node counts are padded to a uniform NPC_PAD so all 8 cores run one SPMD
    program.
  - State h is kept feature-major in SBUF: HT [128 feat, NPC_PAD nodes] f32.
  - Per iteration:
      Phase A (aggregate): neigh.T = sum_{edges} h[src] via
        dma_gather of bf16 h rows (messages, token-major) from an AllGathered
        DRAM table + one-hot PE matmuls (lhsT=msg chunk, rhs=host-precomputed
        one-hot tile streamed from DRAM) accumulating per 128-node dst block
        in PSUM, then DVE copy/add into a bf16 SBUF accumulator ACC.
        Edges are grouped (stream k = table-row range for int16 gather idxs,
        dst block b) with a shared cross-core schedule (max chunk counts).
      Phase B (conv+update): per block: PSUM = aW.T-matmul(HT) +
        lin_w-matmul(ACC); HT += eps*tanh(PSUM + bias) (ACT+DVE); node-major
        bf16 h rows written via PE transpose -> staging -> DRAM; AllGather
        rebuilds the global table (skipped after the last iteration).
  - Pooling (all core-local, no collective): sum via batch-one-hot matmuls on
    the node-major staging tiles; max via a -inf-padded dma_gather + two-level
    reduce (free-dim reduce, PE transpose, reduce); mean = sum * 1/cnt.
  - Readout MLP on-device per core ([16 graphs, 16] out); host concatenates.
"""

import sys

if "/opt/trn_rl_repo" not in sys.path:
    sys.path.insert(0, "/opt/trn_rl_repo")

import numpy as np
import ml_dtypes

import concourse.bacc as bacc
import concourse.bass as bass
import concourse.mybir as mybir
import concourse.tile as tile
from concourse.bass_utils import run_bass_kernel_spmd

F32 = mybir.dt.float32
BF16 = mybir.dt.bfloat16
I16 = mybir.dt.int16

NCORES = 8
N_GRAPHS = 128
GPC = N_GRAPHS // NCORES  # graphs per core
IN_DIM, HID, OUT_DIM = 64, 128, 16
NUM_ITERS = 4
GAMMA, EPS = 0.1, 0.1
CALL_COLS = 16          # max gather-call size in 128-token columns
NQ = 4                  # SWDGE queues
EMB_CHUNK = 512
NEG_BIG = -1.0e30


def _ceil_to(x, m):
    return -(-x // m) * m


# ---------------------------------------------------------------- host plan

class Plan:
    pass


def build_plan(edge_index, batch):
    """All host-side preprocessing. Returns a Plan with the shared schedule
    and per-core input arrays."""
    p = Plan()
    n_nodes = batch.shape[0]
    src = np.asarray(edge_index[0], dtype=np.int64)
    dst = np.asarray(edge_index[1], dtype=np.int64)
    batch = np.asarray(batch, dtype=np.int64)
    assert (np.diff(batch) >= 0).all(), "batch must be sorted"

    # graph -> core, node -> core
    graph_start = np.searchsorted(batch, np.arange(N_GRAPHS + 1))  # [129]
    core_start = graph_start[:: GPC]  # [9] node index where each core starts
    n_c = np.diff(core_start)  # nodes per core
    npc_pad = int(_ceil_to(max(int(n_c.max()), EMB_CHUNK), EMB_CHUNK))
    assert 2 * npc_pad <= 32767
    p.npc_pad = npc_pad
    p.nblk = npc_pad // 128
    p.ss = 2 * npc_pad            # stream size in table rows
    p.nstream = 4
    p.core_start = core_start
    p.n_c = n_c

    node_core = (batch // GPC).astype(np.int64)
    node_local = np.arange(n_nodes, dtype=np.int64) - core_start[node_core]
    table_row = node_core * npc_pad + node_local  # global table row per node

    # ---- edge grouping
    e_core = node_core[dst]                      # owner core (by dst)
    e_tr = table_row[src]
    e_k = e_tr // p.ss                           # stream
    e_gidx = (e_tr - e_k * p.ss).astype(np.int16)
    e_ld = node_local[dst]
    e_b = e_ld // 128                            # dst block
    e_slot = (e_ld % 128).astype(np.int64)

    nblk = p.nblk
    key = (e_core * p.nstream + e_k) * nblk + e_b
    order = np.argsort(key * np.int64(40000) + e_gidx, kind="stable")
    key_s = key[order]
    counts = np.bincount(key_s, minlength=NCORES * p.nstream * nblk)
    counts = counts.reshape(NCORES, p.nstream, nblk)

    chunk_counts = (-(-counts // 128)).max(axis=0)  # [nstream, nblk]
    chunk_counts[0] = np.maximum(chunk_counts[0], 1)
    p.chunk_counts = chunk_counts
    # column offsets, k-major then b
    flat = chunk_counts.reshape(-1)
    col_off = np.zeros_like(flat)
    col_off[1:] = np.cumsum(flat)[:-1]
    col_off = col_off.reshape(p.nstream, nblk)
    p.col_off = col_off
    p.c_tot = int(flat.sum())
    p.stream_col0 = [int(col_off[k, 0]) for k in range(p.nstream)]
    p.stream_cols = [int(chunk_counts[k].sum()) for k in range(p.nstream)]

    # ---- gather calls: pack consecutive (k,b) runs, <= CALL_COLS cols each
    calls = []  # (k, col0, ncols, [(b, local_col, cc), ...])
    for k in range(p.nstream):
        cur = None
        for b in range(nblk):
            cc = int(chunk_counts[k, b])
            if cc == 0:
                continue
            c0 = int(col_off[k, b])
            if cur is None or cur[1] + cur[2] != c0 or cur[2] + cc > CALL_COLS:
                if cur is not None:
                    calls.append(cur)
                cur = [k, c0, 0, []]
            cur[3].append((b, cur[2], cc))
            cur[2] += cc
        if cur is not None:
            calls.append(cur)
    p.calls = [tuple(c) for c in calls]

    # ---- per-core token data
    tok_tot = p.c_tot * 128
    p.tok_tot = tok_tot
    seg_start_in_sorted = np.zeros(NCORES * p.nstream * nblk + 1, np.int64)
    seg_start_in_sorted[1:] = np.cumsum(counts.reshape(-1))
    within = np.arange(len(order), dtype=np.int64) - seg_start_in_sorted[key_s]
    tokpos = (col_off.reshape(-1)[key_s % (p.nstream * nblk)] * 128 + within)

    p.msgidx = []
    p.slots = []
    for c in range(NCORES):
        mask = (key_s // (p.nstream * nblk)) == c
        tp = tokpos[mask]
        gi = e_gidx[order][mask]
        sl = e_slot[order][mask]
        arr16 = np.zeros((16, tok_tot // 16), np.int16)
        arr16[tp % 16, tp // 16] = gi
        p.msgidx.append(np.tile(arr16, (8, 1)))
        oh = np.zeros((128, p.c_tot, 128), ml_dtypes.bfloat16)
        oh[tp % 128, tp // 128, sl] = 1.0
        p.slots.append(oh)

    # ---- pooling
    gsz = np.diff(graph_start)
    p.k_pool = int(_ceil_to(max(int(gsz.max()), 128), 128))
    pool_cols_per_graph = p.k_pool // 128
    graphs_per_call = max(1, CALL_COLS // pool_cols_per_graph)
    p.pool_graphs_per_call = graphs_per_call
    p.pool_calls = []
    g = 0
    while g < GPC:
        ng = min(graphs_per_call, GPC - g)
        p.pool_calls.append((g, ng))
        g += ng
    pool_tok = GPC * p.k_pool
    p.pool_tok = pool_tok

    p.poolidx = []
    p.poolhot = []
    p.invcnt = []
    for c in range(NCORES):
        idx = np.full(pool_tok, npc_pad, np.int64)  # default: -inf pad row
        for j in range(GPC):
            g_id = c * GPC + j
            s = graph_start[g_id] - core_start[c]
            e = graph_start[g_id + 1] - core_start[c]
            idx[j * p.k_pool: j * p.k_pool + (e - s)] = np.arange(s, e)
        arr16 = np.zeros((16, pool_tok // 16), np.int16)
        t = np.arange(pool_tok)
        arr16[t % 16, t // 16] = idx.astype(np.int16)
        p.poolidx.append(np.tile(arr16, (8, 1)))

        ph = np.zeros((128, p.nblk, GPC), ml_dtypes.bfloat16)
        ln = np.arange(int(n_c[c]), dtype=np.int64)
        gslot = batch[core_start[c]: core_start[c + 1]] - c * GPC
        ph[ln % 128, ln // 128, gslot] = 1.0
        p.poolhot.append(ph)

        cnt = gsz[c * GPC: (c + 1) * GPC].astype(np.float32)
        inv = 1.0 / np.maximum(cnt, 1.0)
        p.invcnt.append(np.tile(inv[None, :], (128, 1)).astype(np.float32))

    return p


def prepare_inputs(p, x, emb_w, emb_b, W, asym_b, lin_w, r1_w, r1_b, r2_w, r2_b):
    """Build per-core in_maps."""
    aW = W - W.T - GAMMA * np.eye(HID, dtype=np.float32)
    shared = {
        "embWT": np.ascontiguousarray(emb_w.T.astype(np.float32)),
        "embB": emb_b.astype(np.float32).reshape(128, 1),
        "aWT": np.ascontiguousarray(aW.T.astype(np.float32)),
        "linWT": np.ascontiguousarray(lin_w.T).astype(ml_dtypes.bfloat16),
        "asymB": asym_b.astype(np.float32).reshape(128, 1),
        "ident": np.eye(128, dtype=np.float32),
        "r1wt_add": np.ascontiguousarray(r1_w[:, 0:128].T.astype(np.float32)),
        "r1wt_max": np.ascontiguousarray(r1_w[:, 128:256].T.astype(np.float32)),
        "r1wt_mean": np.ascontiguousarray(r1_w[:, 256:384].T.astype(np.float32)),
        "r1b_a": r1_b[0:128].astype(np.float32).reshape(128, 1),
        "r1b_b": r1_b[128:192].astype(np.float32).reshape(64, 1),
        "r2wt_a": np.ascontiguousarray(r2_w[:, 0:128].T.astype(np.float32)),
        "r2wt_b": np.ascontiguousarray(r2_w[:, 128:192].T.astype(np.float32)),
        "r2b": np.tile(r2_b.astype(np.float32).reshape(1, 16), (GPC, 1)),
    }
    in_maps = []
    for c in range(NCORES):
        s, n = int(p.core_start[c]), int(p.n_c[c])
        xT = np.zeros((IN_DIM, p.npc_pad), np.float32)
        xT[:, :n] = x[s: s + n].T
        m = dict(shared)
        m["xT"] = xT
        m["msgidx"] = p.msgidx[c]
        m["onehot"] = p.slots[c]
        m["poolidx"] = p.poolidx[c]
        m["poolhot"] = np.ascontiguousarray(
            p.poolhot[c].reshape(128, p.nblk * GPC))
        m["invcnt"] = p.invcnt[c]
        in_maps.append(m)
    return in_maps


# ---------------------------------------------------------------- device

def build_program(p, stage=99):
    nc = bacc.Bacc("TRN2", num_devices=NCORES, num_swdge_queues=NQ,
                   debug=False)
    npc = p.npc_pad
    nblk = p.nblk

    # I/O
    d_xT = nc.dram_tensor("xT", [IN_DIM, npc], F32, kind="ExternalInput")
    d_embWT = nc.dram_tensor("embWT", [IN_DIM, HID], F32, kind="ExternalInput")
    d_embB = nc.dram_tensor("embB", [HID, 1], F32, kind="ExternalInput")
    d_aWT = nc.dram_tensor("aWT", [HID, HID], F32, kind="ExternalInput")
    d_linWT = nc.dram_tensor("linWT", [HID, HID], BF16, kind="ExternalInput")
    d_asymB = nc.dram_tensor("asymB", [HID, 1], F32, kind="ExternalInput")
    d_ident = nc.dram_tensor("ident", [128, 128], F32, kind="ExternalInput")
    d_msgidx = nc.dram_tensor("msgidx", [128, p.tok_tot // 16], I16,
                              kind="ExternalInput")
    d_onehot = nc.dram_tensor("onehot", [128, p.c_tot, 128], BF16,
                              kind="ExternalInput")
    d_poolidx = nc.dram_tensor("poolidx", [128, p.pool_tok // 16], I16,
                               kind="ExternalInput")
    d_poolhot = nc.dram_tensor("poolhot", [128, nblk * GPC], BF16,
                               kind="ExternalInput")
    d_invcnt = nc.dram_tensor("invcnt", [128, GPC], F32, kind="ExternalInput")
    d_r1wt = [nc.dram_tensor(nm, [128, 192], F32, kind="ExternalInput")
              for nm in ("r1wt_add", "r1wt_max", "r1wt_mean")]
    d_r1b_a = nc.dram_tensor("r1b_a", [128, 1], F32, kind="ExternalInput")
    d_r1b_b = nc.dram_tensor("r1b_b", [64, 1], F32, kind="ExternalInput")
    d_r2wt_a = nc.dram_tensor("r2wt_a", [128, 16], F32, kind="ExternalInput")
    d_r2wt_b = nc.dram_tensor("r2wt_b", [64, 16], F32, kind="ExternalInput")
    d_r2b = nc.dram_tensor("r2b", [GPC, 16], F32, kind="ExternalInput")
    d_out = nc.dram_tensor("out", [GPC, OUT_DIM], F32, kind="ExternalOutput")
    d_dbg = nc.dram_tensor("dbg", [128, 1024], F32, kind="ExternalOutput")

    # internal DRAM
    d_stage = nc.dram_tensor("stage", [npc + 128, HID], BF16, kind="Internal")
    d_table = nc.dram_tensor("table", [NCORES * npc, HID], BF16,
                             kind="Internal", addr_space="Shared")

    Tanh = mybir.ActivationFunctionType.Tanh
    Lrelu = mybir.ActivationFunctionType.Lrelu
    Ident = mybir.ActivationFunctionType.Identity
    ADD = mybir.AluOpType.add
    MAX = mybir.AluOpType.max
    MULT = mybir.AluOpType.mult
    X = mybir.AxisListType.X
    rg = [list(range(NCORES))]

    with tile.TileContext(nc) as tc:
        with tc.tile_pool(name="const", bufs=1) as cst, \
             tc.tile_pool(name="state", bufs=1) as st, \
             tc.tile_pool(name="msg", bufs=10) as msgp, \
             tc.tile_pool(name="oh", bufs=4) as ohp, \
             tc.tile_pool(name="idx", bufs=2) as idxp, \
             tc.tile_pool(name="wrk", bufs=4) as wrk, \
             tc.tile_pool(name="stg", bufs=2) as stgp, \
             tc.tile_pool(name="psA", bufs=3, space="PSUM") as psA, \
             tc.tile_pool(name="psB", bufs=2, space="PSUM") as psB, \
             tc.tile_pool(name="psT", bufs=2, space="PSUM") as psT, \
             tc.tile_pool(name="psP", bufs=1, space="PSUM") as psP:

            # ---- load constants
            embWT = cst.tile([IN_DIM, HID], F32)
            embB = cst.tile([HID, 1], F32)
            aWT = cst.tile([HID, HID], F32)
            linWT = cst.tile([HID, HID], BF16)
            asymB = cst.tile([HID, 1], F32)
            ident = cst.tile([128, 128], F32)
            poolhot = cst.tile([128, nblk * GPC], BF16)
            invcnt = cst.tile([128, GPC], F32)
            nc.sync.dma_start(embWT[:], d_embWT[:])
            nc.sync.dma_start(embB[:], d_embB[:])
            nc.sync.dma_start(aWT[:], d_aWT[:])
            nc.sync.dma_start(linWT[:], d_linWT[:])
            nc.sync.dma_start(asymB[:], d_asymB[:])
            nc.sync.dma_start(ident[:], d_ident[:])
            nc.sync.dma_start(poolhot[:], d_poolhot[:])
            nc.sync.dma_start(invcnt[:], d_invcnt[:])

            HT = st.tile([HID, npc], F32)
            ACC = st.tile([HID, npc], BF16)
            idx_res = st.tile([128, p.tok_tot // 16], I16)
            nc.sync.dma_start(idx_res[:], d_msgidx[:])

            # -inf pad row for max-pool gather
            minf = wrk.tile([1, HID], BF16, tag="minf")
            nc.vector.memset(minf[:], NEG_BIG)
            nc.sync.dma_start(d_stage[npc: npc + 1, :], minf[:])

            # ---- embedding: HT = emb_w @ x.T + emb_b
            for j in range(npc // EMB_CHUNK):
                xt = wrk.tile([IN_DIM, EMB_CHUNK], F32, tag="xt")
                nc.sync.dma_start(
                    xt[:], d_xT[:, j * EMB_CHUNK: (j + 1) * EMB_CHUNK])
                pse = psB.tile([HID, EMB_CHUNK], F32, tag="conv")
                nc.tensor.matmul(pse[:], embWT[:], xt[:], start=True, stop=True)
                nc.scalar.activation(
                    HT[:, j * EMB_CHUNK: (j + 1) * EMB_CHUNK], pse[:],
                    Ident, bias=embB[:])

            def table_write(last_iter):
                """Transpose HT -> node-major bf16 staging -> stage dram
                (+ pool-sum matmuls on the staging tiles in the last iter).
                Returns the pool psum tile in the last iter."""
                pool_ps = None
                if last_iter:
                    pool_ps = psP.tile([HID, GPC], F32, tag="poolps")
                nst = 8  # blocks per staging tile
                for b0 in range(0, nblk, nst):
                    nb = min(nst, nblk - b0)
                    stg = stgp.tile([128, nst * HID], BF16, tag="stage")
                    for i in range(nb):
                        b = b0 + i
                        trp = psT.tile([128, HID], F32, tag="tr")
                        nc.tensor.transpose(
                            trp[:], HT[:, b * 128: (b + 1) * 128], ident[:])
                        nc.vector.tensor_copy(
                            stg[:, i * HID: (i + 1) * HID], trp[:])
                        if last_iter:
                            nc.tensor.matmul(
                                pool_ps[:], stg[:, i * HID: (i + 1) * HID],
                                poolhot[:, b * GPC: (b + 1) * GPC],
                                start=(b == 0), stop=(b == nblk - 1),
                                skip_group_check=True)
                    nc.sync.dma_start(
                        d_stage[b0 * 128: b0 * 128 + nb * 128, :].rearrange(
                            "(a p) f -> p a f", p=128),
                        stg[:, : nb * HID].rearrange(
                            "p (a f) -> p a f", f=HID))
                return pool_ps

            if stage >= 2:
                table_write(False)
                nc.gpsimd.collective_compute(
                    "AllGather", mybir.AluOpType.bypass, replica_groups=rg,
                    ins=[d_stage[0:npc, :].opt()], outs=[d_table[:].opt()])

            pool_ps = None
            n_iters = 0 if stage < 3 else (1 if stage < 5 or 30 < stage < 40 else NUM_ITERS)
            for it in range(n_iters):
                # ---------------- phase A: aggregate into ACC
                for ci, (k, c0, ncols, runs) in enumerate(p.calls):
                    ntok = ncols * 128
                    msgt = msgp.tile([128, CALL_COLS, HID], BF16, tag="msg")
                    nc.gpsimd.dma_gather(
                        msgt[:, :ncols, :],
                        d_table[k * p.ss: (k + 1) * p.ss, :],
                        idx_res[:, c0 * 8: c0 * 8 + ncols * 8], ntok, ntok,
                        HID, single_packet=False, queue_num=ci % NQ)
                    if stage == 31:
                        nc.vector.tensor_copy(
                            ACC[:, 0:HID], msgt[:, 0, :])
                        continue
                    oht = ohp.tile([128, CALL_COLS, HID], BF16, tag="oh")
                    oh_eng = nc.scalar if ci % 2 else nc.sync
                    oh_eng.dma_start(
                        oht[:, :ncols, :], d_onehot[:, c0: c0 + ncols, :])
                    if stage == 32:
                        nc.vector.tensor_copy(ACC[:, 0:HID], oht[:, 0, :])
                        continue
                    for (b, lc, cc) in runs:
                        ps = psA.tile([HID, 128], F32, tag="agg")
                        for j in range(cc):
                            nc.tensor.matmul(
                                ps[:], msgt[:, lc + j, :], oht[:, lc + j, :],
                                start=(j == 0), stop=(j == cc - 1))
                        if stage == 33:
                            continue
                        sl = ACC[:, b * 128: (b + 1) * 128]
                        if k == 0:
                            nc.vector.tensor_copy(sl, ps[:])
                        else:
                            nc.vector.tensor_tensor(sl, ps[:], sl, ADD)

                # ---------------- phase B: conv + update
                for b in range(nblk if stage >= 4 else 0):
                    ps = psB.tile([HID, 128], F32, tag="conv")
                    nc.tensor.matmul(ps[:], aWT[:],
                                     HT[:, b * 128: (b + 1) * 128],
                                     start=True, stop=False)
                    nc.tensor.matmul(ps[:], linWT[:],
                                     ACC[:, b * 128: (b + 1) * 128],
                                     start=False, stop=True)
                    th = wrk.tile([HID, 128], F32, tag="tanh")
                    nc.scalar.activation(th[:], ps[:], Tanh, bias=asymB[:])
                    sl = HT[:, b * 128: (b + 1) * 128]
                    nc.vector.scalar_tensor_tensor(sl, th[:], EPS, sl,
                                                   MULT, ADD)
                if stage < 4:
                    continue
                pool_ps = table_write(it == n_iters - 1)
                if it < n_iters - 1:
                    nc.gpsimd.collective_compute(
                        "AllGather", mybir.AluOpType.bypass, replica_groups=rg,
                        ins=[d_stage[0:npc, :].opt()],
                        outs=[d_table[:].opt()])

            # ---------------- debug dumps for staged runs
            if stage < 7:
                w = min(512, npc)
                dbg_t = wrk.tile([128, 1024], F32, tag="dbg")
                nc.vector.memset(dbg_t[:], 0.0)
                if stage >= 3 and n_iters > 0:
                    nc.vector.tensor_copy(dbg_t[:, 0:w], ACC[:, 0:w])
                    nc.vector.tensor_copy(dbg_t[:, 512:512 + w], HT[:, 0:w])
                elif stage == 2:
                    tabt = wrk.tile([128, HID], BF16, tag="tab_dbg")
                    nc.sync.dma_start(tabt[:], d_table[0:128, :])
                    nc.vector.tensor_copy(dbg_t[:, 0:HID], tabt[:])
                    nc.vector.tensor_copy(dbg_t[:, 512:512 + w], HT[:, 0:w])
                else:
                    nc.vector.tensor_copy(dbg_t[:, 0:w], HT[:, 0:w])
                nc.sync.dma_start(d_dbg[:], dbg_t[:])

            do_tail = stage >= 6 or stage == 61
            if do_tail:
                # ---------------- pooling
                poolsum = wrk.tile([HID, GPC], F32, tag="psum_sb")
                nc.vector.tensor_copy(poolsum[:], pool_ps[:])
                poolmean = wrk.tile([HID, GPC], F32, tag="pmean_sb")
                nc.vector.tensor_tensor(poolmean[:], poolsum[:], invcnt[:], MULT)
                poolmax = wrk.tile([HID, GPC], F32, tag="pmax_sb")

                cols_per_g = p.k_pool // 128
                for (g0, ng) in p.pool_calls:
                    ncols = ng * cols_per_g
                    ntok = ncols * 128
                    t0 = g0 * p.k_pool
                    idxt = idxp.tile([128, CALL_COLS * 8], I16, tag="idx")
                    nc.sync.dma_start(
                        idxt[:, : ncols * 8],
                        d_poolidx[:, t0 // 16: t0 // 16 + ncols * 8])
                    gat = msgp.tile([128, CALL_COLS, HID], BF16, tag="msg")
                    nc.gpsimd.dma_gather(
                        gat[:, :ncols, :], d_stage[:, :],
                        idxt[:, : ncols * 8], ntok, ntok, HID,
                        single_packet=False, queue_num=(g0 // 4) % NQ)
                    for j in range(ng):
                        g = g0 + j
                        part = wrk.tile([128, HID], F32, tag="mpart")
                        nc.vector.tensor_reduce(
                            part[:],
                            gat[:, j * cols_per_g: (j + 1) * cols_per_g, :]
                            .rearrange("p c f -> p f c"),
                            X, MAX)
                        trp = psT.tile([128, HID], F32, tag="tr")
                        nc.tensor.transpose(trp[:], part[:], ident[:])
                        nc.vector.tensor_reduce(
                            poolmax[:, g: g + 1], trp[:], X, MAX)

                if stage == 6:
                    dbg_t = wrk.tile([128, 1024], F32, tag="dbg")
                    nc.vector.memset(dbg_t[:], 0.0)
                    nc.vector.tensor_copy(dbg_t[:, 0:GPC], poolsum[:])
                    nc.vector.tensor_copy(dbg_t[:, 64:64 + GPC], poolmax[:])
                    nc.vector.tensor_copy(dbg_t[:, 128:128 + GPC], poolmean[:])
                    nc.sync.dma_start(d_dbg[:], dbg_t[:])
                # ---------------- readout MLP
                r1wt = []
                for d in d_r1wt:
                    t = cst.tile([128, 192], F32)
                    nc.sync.dma_start(t[:], d[:])
                    r1wt.append(t)
                r1b_a = cst.tile([128, 1], F32)
                r1b_b = cst.tile([64, 1], F32)
                r2wt_a = cst.tile([128, 16], F32)
                r2wt_b = cst.tile([64, 16], F32)
                r2bb = cst.tile([GPC, 16], F32)
                nc.sync.dma_start(r1b_a[:], d_r1b_a[:])
                nc.sync.dma_start(r1b_b[:], d_r1b_b[:])
                nc.sync.dma_start(r2wt_a[:], d_r2wt_a[:])
                nc.sync.dma_start(r2wt_b[:], d_r2wt_b[:])
                nc.sync.dma_start(r2bb[:], d_r2b[:])

                g1 = []
                for (m0, msz, bt) in ((0, 128, r1b_a), (128, 64, r1b_b)):
                    psr = psB.tile([msz, GPC], F32, tag="conv")
                    for wi, src_t in ((0, poolsum), (1, poolmax), (2, poolmean)):
                        nc.tensor.matmul(psr[:], r1wt[wi][:, m0: m0 + msz],
                                         src_t[:], start=(wi == 0), stop=(wi == 2))
                    gt = wrk.tile([msz, GPC], F32, tag=f"g1_{m0}")
                    nc.scalar.activation(gt[:], psr[:], Ident, bias=bt[:])
                    nc.vector.scalar_tensor_tensor(gt[:], gt[:], 0.01, gt[:],
                                                   MULT, MAX)
                    g1.append(gt)

                if stage == 61:
                    dbg_t = wrk.tile([128, 1024], F32, tag="dbg")
                    nc.vector.memset(dbg_t[:], 0.0)
                    nc.vector.tensor_copy(dbg_t[:, 0:GPC], g1[0][:])
                    nc.vector.tensor_copy(dbg_t[0:64, 64:64 + GPC], g1[1][:])
                    nc.sync.dma_start(d_dbg[:], dbg_t[:])
                ps2a = psB.tile([GPC, OUT_DIM], F32, tag="conv")
                nc.tensor.matmul(ps2a[:], g1[0][:, :], r2wt_a[:],
                                 start=True, stop=True)
                ps2b = psT.tile([GPC, OUT_DIM], F32, tag="tr")
                nc.tensor.matmul(ps2b[:], g1[1][:, :], r2wt_b[:],
                                 start=True, stop=True)
                t2a = wrk.tile([GPC, OUT_DIM], F32, tag="t2a")
                nc.scalar.copy(t2a[:], ps2a[:])
                t2b = wrk.tile([GPC, OUT_DIM], F32, tag="t2b")
                nc.vector.tensor_tensor(t2b[:], ps2b[:], t2a[:], ADD)
                outt = wrk.tile([GPC, OUT_DIM], F32, tag="outt")
                nc.vector.tensor_tensor(outt[:], t2b[:], r2bb[:], ADD)
                nc.vector.scalar_tensor_tensor(outt[:], outt[:], 0.01, outt[:],
                                               MULT, MAX)
                nc.sync.dma_start(d_out[:], outt[:])

    nc.compile()
    return nc


# ---------------------------------------------------------------- entry

_CACHE = {}


def _run(inputs, trace=False, stage=99):
    x = np.asarray(inputs["x"], np.float32)
    edge_index = np.asarray(inputs["edge_index"])
    batch = np.asarray(inputs["batch"])
    plan_key = (edge_index.tobytes(), batch.tobytes(), stage)
    key = hash(plan_key)
    if key in _CACHE:
        p, nc = _CACHE[key]
    else:
        p = build_plan(edge_index, batch)
        nc = build_program(p, stage=stage)
        _CACHE[key] = (p, nc)

    in_maps = prepare_inputs(
        p, x,
        np.asarray(inputs["emb_w"], np.float32),
        np.asarray(inputs["emb_b"], np.float32),
        np.asarray(inputs["W"], np.float32),
        np.asarray(inputs["asym_b"], np.float32),
        np.asarray(inputs["lin_w"], np.float32),
        np.asarray(inputs["r1_w"], np.float32),
        np.asarray(inputs["r1_b"], np.float32),
        np.asarray(inputs["r2_w"], np.float32),
        np.asarray(inputs["r2_b"], np.float32),
    )
    res = run_bass_kernel_spmd(nc, in_maps, core_ids=list(range(NCORES)),
                               trace=trace)
    out = np.concatenate([res.results[c]["out"] for c in range(NCORES)], 0)
    return out.astype(np.float32), res


def kernel(**inputs):
    out, _ = _run(inputs, trace=False)
    return out

